# revision 5
# baseline (speedup 1.0000x reference)
"""Trainium2 Bass kernel for nn_LinearNet (complex double-linear).

Reference math (N = 4096):
    R_r = x @ W_r^T          R_i = x @ W_i^T
    C_r = W_r^T @ R_r - W_i^T @ R_i
    C_i = W_r^T @ R_i + W_i^T @ R_r
    out = concat([C_r, C_i], axis=1)                    # [N, 2N]

Sharding: core c owns output columns S_c = [c*512, (c+1)*512) of both C_r
and C_i.  No inter-core communication.

Pass 2 uses the 3-multiplication complex trick (Karatsuba):
    t1 = W_r^T R_r,  t2 = W_i^T R_i,  t3 = (W_r+W_i)^T (R_r+R_i)
    C_r = t1 - t2,   C_i = t3 - t1 - t2
which cuts the total matmul volume from 6 to 5 N^3-units.  All matmul
operands are bf16 (full PE speed, half the DMA/SBUF of f32); PSUM
accumulation stays fp32.  R never leaves SBUF: pass 1 evacuates PSUM
straight into the bf16 SBUF tiles pass 2 consumes, and R_s = R_r + R_i
is formed on the Pool engine during pass 1.

W_s = W_r + W_i is formed on the Vector engine during pass 2 (saves a
third 32 MB HBM weight stream and halves the per-ring DMA load of
pass 2).  Host-side layout prep turns every device DMA into a
contiguous slab.  Pass 2's first weight half-chunks are prefetched on
the otherwise-idle GpSimd SWDGE queue during pass 1, keeping the two
HWDGE rings' trigger slots (the framework round-robins ~10 completion
semaphores; excess in-flight transfers serialize trigger issue) for
the warm-up x/weight stream.  A burst of zero matmuls right after the
NEFF preamble holds the PE HAM clock-gate at 8/8 until the first real
operands land, so no real matmul runs at the cold 1.2 GHz rate.
"""

import numpy as np

N = 4096
P = 128
NCORES = 8
SH = N // NCORES  # 512 output columns per core
KT = N // P  # 32 contraction tiles
MSUP = 256  # pass-1 output-row super tile (2 PSUM pairs live)
JH = KT // 2  # pass-2 j-tiles per weight half-slab

_CACHE = {}


def _build():
    import concourse.mybir as mybir
    import concourse.tile as tile
    from concourse import bacc

    f32 = mybir.dt.float32
    bf16 = mybir.dt.bfloat16
    sub = mybir.AluOpType.subtract
    add = mybir.AluOpType.add

    nc = bacc.Bacc()
    # xP[ms*128+p, k*256+c] = x[ms*256+c, k*128+p]
    xP = nc.declare_dram_parameter("xP", [N // MSUP * P, KT * MSUP], bf16,
                                   isOutput=False)
    # wrT[p, k*512+n] = W_r[c*512+n, k*128+p]   (per-core slice)
    wrT = nc.declare_dram_parameter("wrT", [P, KT * SH], bf16, isOutput=False)
    wiT = nc.declare_dram_parameter("wiT", [P, KT * SH], bf16, isOutput=False)
    # wrP[a*128+p, j*128+c] = W_r[j*128+p, a*128+c]   (tile-transposed full W)
    wrP = nc.declare_dram_parameter("wrP", [N, N], bf16, isOutput=False)
    wiP = nc.declare_dram_parameter("wiP", [N, N], bf16, isOutput=False)
    out_r = nc.declare_dram_parameter("out_r", [N, SH], f32, isOutput=True)
    out_i = nc.declare_dram_parameter("out_i", [N, SH], f32, isOutput=True)

    with tile.TileContext(nc) as tc:
        # ~6.5 us of tiny zero matmuls starting right after the NEFF
        # preamble: the PE HAM activity window stays busy until the first
        # real operands arrive (~15 us), so the real warm-up matmuls run
        # at the full 2.4 GHz clock instead of the cold 1.2 GHz rate.
        with (
            tc.tile_pool(name="warm", bufs=1) as wp,
            tc.tile_pool(name="warmp", bufs=1, space="PSUM") as wpp,
        ):
            wsb = wp.tile([P, P], bf16)
            nc.vector.memset(wsb[:], 0)
            wps = wpp.tile([P, P], f32)
            NWARM = 45
            for i in range(NWARM):
                nc.tensor.matmul(wps[:], wsb[:], wsb[:],
                                 start=i == 0, stop=i == NWARM - 1)

        # R lives in SBUF for the whole kernel: 3 x 32 KB/partition (bf16).
        # wrc0/wic0/wsc0: pass-2 a=0 first weight half-slabs (prefetched on
        # the GpSimd SWDGE queue during pass 1; wsc0 is formed on Vector).
        with tc.tile_pool(name="rres", bufs=1) as r_pool:
            rr_sb = r_pool.tile([P, KT * SH], bf16)
            ri_sb = r_pool.tile([P, KT * SH], bf16)
            rs_sb = r_pool.tile([P, KT * SH], bf16)
            wrc0 = r_pool.tile([P, JH * P], bf16)
            wic0 = r_pool.tile([P, JH * P], bf16)
            wsc0 = r_pool.tile([P, JH * P], bf16)

            # ---------- pass 1: R[:, S_c] = x @ W[S_c, :]^T ----------
            # psum[m, n] = sum_kk x[ms*256+m, k*128+kk] * W[n, k*128+kk]
            with (
                tc.tile_pool(name="wt", bufs=1) as wt_pool,
                tc.tile_pool(name="xs", bufs=3) as xs_pool,
                tc.tile_pool(name="ps1", bufs=4, space="PSUM") as ps1,
            ):
                wrT_sb = wt_pool.tile([P, KT * SH], bf16)
                wiT_sb = wt_pool.tile([P, KT * SH], bf16)
                msub = MSUP // P  # 2
                XH = KT // 2  # k-tiles per x half-tile

                def load_xh(ms, h, name):
                    xh = xs_pool.tile([P, XH * MSUP], bf16, tag="xc",
                                      name=name)
                    nc.sync.dma_start(
                        xh[:],
                        xP[ms * P : (ms + 1) * P,
                           h * XH * MSUP : (h + 1) * XH * MSUP],
                    )
                    return xh

                def wi_chunk(k0, k1):
                    nc.sync.dma_start(wiT_sb[:, k0 * SH : k1 * SH],
                                      wiT[:, k0 * SH : k1 * SH])

                # Weight preload is split across BOTH HWDGE rings so the
                # warm-up's weight demand is fed at 2-queue bandwidth: wrT on
                # the scalar ring, wiT on the sync ring interleaved with the
                # first x half-tiles (ordered so the first matmuls gate on as
                # little data as possible).
                def wr_chunk(k0, k1):
                    nc.scalar.dma_start(wrT_sb[:, k0 * SH : k1 * SH],
                                        wrT[:, k0 * SH : k1 * SH])

                # Startup schedule, need-ordered per ring.  The sync ring
                # (data from ~8.6 us) carries what the solo phase and the
                # early interleave gate on; the scalar ring (data from
                # ~11.3 us) carries the wrT bulk + mid wiT chunks; the
                # GpSimd SWDGE queue takes the latest wiT chunk.  The warm
                # matmuls consume ~240 GB/s once the fused sweep starts, so
                # neither HW ring can carry much more than ~5 MB of the
                # startup set without stalling the PE.
                x00 = load_xh(0, 0, "x00")
                wi_chunk(0, 4)
                x10 = load_xh(1, 0, "x10")
                wi_chunk(8, 16)
                x01 = load_xh(0, 1, "x01")
                wr_chunk(0, 4)
                wr_chunk(4, 8)
                nc.scalar.dma_start(wiT_sb[:, 4 * SH : 8 * SH],
                                    wiT[:, 4 * SH : 8 * SH])
                wr_chunk(8, 16)
                wr_chunk(16, 24)
                nc.scalar.dma_start(wiT_sb[:, 16 * SH : 24 * SH],
                                    wiT[:, 16 * SH : 24 * SH])
                wr_chunk(24, 32)
                nc.gpsimd.dma_start(wiT_sb[:, 24 * SH : 32 * SH],
                                    wiT[:, 24 * SH : 32 * SH])
                # prefetch pass-2 a=0 first half-slabs on the GpSimd SWDGE
                # queue (idle until pass-2 output stores): keeps the two
                # HWDGE rings' completion-semaphore slots free for the
                # warm-up x/weight stream; wsc0 = wrc0 + wic0 on Vector.
                nc.gpsimd.dma_start(wrc0[:], wrP[0:P, 0 : JH * P])
                nc.gpsimd.dma_start(wic0[:], wiP[0:P, 0 : JH * P])
                nc.vector.tensor_tensor(wsc0[:], wrc0[:], wic0[:], add)

                def alloc_acc():
                    acc_r = [
                        ps1.tile([P, SH], f32, tag="ps_r", name=f"accr{_s}")
                        for _s in range(msub)
                    ]
                    acc_i = [
                        ps1.tile([P, SH], f32, tag="ps_i", name=f"acci{_s}")
                        for _s in range(msub)
                    ]
                    return acc_r, acc_i

                def k_step(xh, acc_r, acc_i, k):
                    kk = k % XH
                    first, last = k == 0, k == KT - 1
                    for s in range(msub):
                        lhs = xh[:, kk * MSUP + s * P : kk * MSUP + (s + 1) * P]
                        nc.tensor.matmul(
                            acc_r[s][:], lhs, wrT_sb[:, k * SH : (k + 1) * SH],
                            start=first, stop=last,
                        )
                        nc.tensor.matmul(
                            acc_i[s][:], lhs, wiT_sb[:, k * SH : (k + 1) * SH],
                            start=first, stop=last,
                        )

                def evac(ms, acc_r, acc_i):
                    for s in range(msub):
                        mt = ms * msub + s
                        sl = slice(mt * SH, (mt + 1) * SH)
                        nc.scalar.copy(rr_sb[:, sl], acc_r[s][:])
                        nc.vector.tensor_copy(ri_sb[:, sl], acc_i[s][:])
                        nc.gpsimd.tensor_add(
                            rs_sb[:, sl], rr_sb[:, sl], ri_sb[:, sl]
                        )

                # --- fused warm-up: ms=0,1 share one k-interleaved sweep
                # (8 PSUM banks), halving the weight-DMA demand per unit of
                # compute while the PE ramps and the 8 MB of weights stream
                # in.  ms0 runs k=0..3 solo first so ms1's x half-tile has
                # time to arrive behind it on the sync ring.
                acc0 = alloc_acc()
                acc1 = alloc_acc()
                for k in range(0, 4):
                    k_step(x00, *acc0, k)
                for k in range(0, XH - 4):
                    k_step(x10, *acc1, k)
                    k_step(x00, *acc0, k + 4)
                x11 = load_xh(1, 1, "x11")
                for k in range(XH - 4, XH):
                    k_step(x10, *acc1, k)
                for k in range(XH, KT):
                    k_step(x01, *acc0, k)
                for k in range(XH, KT):
                    k_step(x11, *acc1, k)
                evac(0, *acc0)
                evac(1, *acc1)

                for ms in range(2, N // MSUP):  # 14 regular iterations
                    xh0 = load_xh(ms, 0, "xh0")
                    xh1 = load_xh(ms, 1, "xh1")
                    if ms < N // MSUP - 1:
                        acc_r, acc_i = alloc_acc()
                        for k in range(KT):
                            k_step(xh0 if k < XH else xh1, acc_r, acc_i, k)
                        evac(ms, acc_r, acc_i)
                    else:
                        # final iteration runs as two 1-subtile generations
                        # (2 PSUM banks each, same matmul columns) so six
                        # banks are already free when pass 2's t-groups
                        # start — removes the pass-transition PSUM handoff
                        for half in range(msub):
                            ar = ps1.tile([P, SH], f32, tag="ps_r",
                                          name=f"accrh{half}")
                            ai = ps1.tile([P, SH], f32, tag="ps_i",
                                          name=f"accih{half}")
                            for k in range(KT):
                                xh = xh0 if k < XH else xh1
                                kk = k % XH
                                lhs = xh[:, kk * MSUP + half * P
                                         : kk * MSUP + (half + 1) * P]
                                nc.tensor.matmul(
                                    ar[:], lhs,
                                    wrT_sb[:, k * SH : (k + 1) * SH],
                                    start=k == 0, stop=k == KT - 1,
                                )
                                nc.tensor.matmul(
                                    ai[:], lhs,
                                    wiT_sb[:, k * SH : (k + 1) * SH],
                                    start=k == 0, stop=k == KT - 1,
                                )
                            mt = ms * msub + half
                            sl = slice(mt * SH, (mt + 1) * SH)
                            nc.scalar.copy(rr_sb[:, sl], ar[:])
                            nc.vector.tensor_copy(ri_sb[:, sl], ai[:])
                            nc.gpsimd.tensor_add(
                                rs_sb[:, sl], rr_sb[:, sl], ri_sb[:, sl]
                            )

            # ---------- pass 2: C[:, S_c] = W^T @ R (Karatsuba) ----------
            # t1[a,b] = sum_j wr[j,a] rr[j,b]; t2: wi,ri; t3: ws,rs
            # weight chunks stream as half-slabs (j 0..15 / 16..31);
            # ws = wr + wi is formed on the Vector engine as slabs land,
            # halving each HWDGE ring's pass-2 load (1 MB per a-tile).
            with (
                tc.tile_pool(name="ws", bufs=3) as ws_pool,
                tc.tile_pool(name="ev2", bufs=3) as ev2_pool,
                tc.tile_pool(name="ps2", bufs=2, space="PSUM") as ps2,
            ):
                for a in range(N // P):  # 32
                    asl = slice(a * P, (a + 1) * P)
                    halves = []
                    for hj in range(2):
                        if a == 0 and hj == 0:
                            halves.append((wrc0, wic0, wsc0))
                            continue
                        csl = slice(hj * JH * P, (hj + 1) * JH * P)
                        wrc = ws_pool.tile([P, JH * P], bf16, tag="wrc",
                                           name=f"wrc{hj}")
                        wic = ws_pool.tile([P, JH * P], bf16, tag="wic",
                                           name=f"wic{hj}")
                        wsc = ws_pool.tile([P, JH * P], bf16, tag="wsc",
                                           name=f"wsc{hj}")
                        nc.sync.dma_start(wrc[:], wrP[asl, csl])
                        nc.scalar.dma_start(wic[:], wiP[asl, csl])
                        nc.vector.tensor_tensor(wsc[:], wrc[:], wic[:], add)
                        halves.append((wrc, wic, wsc))
                    # The last a-tile is split into two 256-wide column
                    # groups so the first group's combines + stores overlap
                    # the second group's matmuls — the full-width tail chain
                    # (combine ~2.4 us + store drain) otherwise sits fully
                    # after the final matmul.
                    last_a = a == N // P - 1
                    for cg, cw in ([(0, SH)] if not last_a
                                   else [(0, SH // 2), (1, SH // 2)]):
                        t1 = ps2.tile([P, cw], f32, tag="t1")
                        t2 = ps2.tile([P, cw], f32, tag="t2")
                        t3 = ps2.tile([P, cw], f32, tag="t3")
                        for j in range(KT):
                            first, last = j == 0, j == KT - 1
                            hj, jj = divmod(j, JH)
                            wrc, wic, wsc = halves[hj]
                            wsl = slice(jj * P, (jj + 1) * P)
                            r0 = j * SH + cg * cw
                            rsl = slice(r0, r0 + cw)
                            nc.tensor.matmul(
                                t1[:], wrc[:, wsl], rr_sb[:, rsl],
                                start=first, stop=last,
                            )
                            nc.tensor.matmul(
                                t2[:], wic[:, wsl], ri_sb[:, rsl],
                                start=first, stop=last,
                            )
                            nc.tensor.matmul(
                                t3[:], wsc[:, wsl], rs_sb[:, rsl],
                                start=first, stop=last,
                            )
                        s1 = ev2_pool.tile([P, cw], f32, tag="s1")
                        cr = ev2_pool.tile([P, cw], f32, tag="cr")
                        ci = ev2_pool.tile([P, cw], f32, tag="ci")
                        nc.scalar.copy(s1[:], t1[:])
                        nc.vector.tensor_tensor(cr[:], s1[:], t2[:], sub)
                        nc.vector.tensor_tensor(ci[:], t3[:], s1[:], sub)
                        nc.vector.tensor_tensor(ci[:], ci[:], t2[:], sub)
                        # outputs go on the Pool/SWDGE path: their triggers
                        # wait on cr/ci, and on the strict-FIFO ACT ring that
                        # wait would block the next wic prefetch trigger.
                        # The final group uses the (now idle) sync HWDGE ring
                        # instead — SWDGE descriptor generation (~7 us) would
                        # otherwise sit on the critical path at the tail.
                        oeng = nc.sync if (last_a and cg == 1) else nc.gpsimd
                        osl = slice(cg * cw, cg * cw + cw)
                        oeng.dma_start(out_r[asl, osl], cr[:])
                        oeng.dma_start(out_i[asl, osl], ci[:])

    nc.finalize()
    return nc


def _get_nc():
    if "nc" not in _CACHE:
        _CACHE["nc"] = _build()
    return _CACHE["nc"]


def _prep_inputs(x, W_r, W_i):
    from ml_dtypes import bfloat16

    x = np.asarray(x, dtype=np.float32)
    Wr = np.asarray(W_r, dtype=np.float32)
    Wi = np.asarray(W_i, dtype=np.float32)

    # xP[ms*128+p, k*256+c] = x[ms*256+c, k*128+p]
    xP = np.ascontiguousarray(
        x.reshape(N // MSUP, MSUP, KT, P).transpose(0, 3, 2, 1)
        .reshape(N // MSUP * P, KT * MSUP)
    ).astype(bfloat16)

    # wP[a*128+p, j*128+c] = W[j*128+p, a*128+c]
    def p2(W):
        return np.ascontiguousarray(
            W.reshape(KT, P, KT, P).transpose(2, 1, 0, 3).reshape(N, N)
        ).astype(bfloat16)

    # wT_c[p, k*512+n] = W[c*512+n, k*128+p]
    def p1(W, c):
        blk = W[c * SH : (c + 1) * SH, :].T  # [4096 (k), 512 (n)]
        return np.ascontiguousarray(
            blk.reshape(KT, P, SH).transpose(1, 0, 2).reshape(P, KT * SH)
        ).astype(bfloat16)

    wrP, wiP = p2(Wr), p2(Wi)
    in_maps = []
    for c in range(NCORES):
        in_maps.append(
            {
                "xP": xP,
                "wrT": p1(Wr, c),
                "wiT": p1(Wi, c),
                "wrP": wrP,
                "wiP": wiP,
            }
        )
    return in_maps


def kernel(x, W_r, W_i, **run_kwargs):
    from concourse.bass_utils import run_bass_kernel_spmd

    nc = _get_nc()
    in_maps = _prep_inputs(x, W_r, W_i)
    out = run_bass_kernel_spmd(nc, in_maps, list(range(NCORES)), **run_kwargs)
    res = out.results

    full = np.empty((N, 2 * N), dtype=np.float32)
    for c in range(NCORES):
        full[:, c * SH : (c + 1) * SH] = res[c]["out_r"]
        full[:, N + c * SH : N + (c + 1) * SH] = res[c]["out_i"]
    if run_kwargs:
        _CACHE["last_result"] = out
    return full



# revision 6
# speedup vs baseline: 1.0101x; 1.0101x over previous
"""Trainium2 Bass kernel for nn_LinearNet (complex double-linear).

Reference math (N = 4096):
    R_r = x @ W_r^T          R_i = x @ W_i^T
    C_r = W_r^T @ R_r - W_i^T @ R_i
    C_i = W_r^T @ R_i + W_i^T @ R_r
    out = concat([C_r, C_i], axis=1)                    # [N, 2N]

Sharding: core c owns output columns S_c = [c*512, (c+1)*512) of both C_r
and C_i.  No inter-core communication.

Pass 2 uses the 3-multiplication complex trick (Karatsuba):
    t1 = W_r^T R_r,  t2 = W_i^T R_i,  t3 = (W_r+W_i)^T (R_r+R_i)
    C_r = t1 - t2,   C_i = t3 - t1 - t2
which cuts the total matmul volume from 6 to 5 N^3-units.  All matmul
operands are bf16 (full PE speed, half the DMA/SBUF of f32); PSUM
accumulation stays fp32.  R never leaves SBUF: pass 1 evacuates PSUM
straight into the bf16 SBUF tiles pass 2 consumes, and R_s = R_r + R_i
is formed on the Pool engine during pass 1.

W_s = W_r + W_i is formed on the Vector engine during pass 2 (saves a
third 32 MB HBM weight stream and halves the per-ring DMA load of
pass 2).  Host-side layout prep turns every device DMA into a
contiguous slab.  Pass 2's first weight half-chunks are prefetched on
the otherwise-idle GpSimd SWDGE queue during pass 1, keeping the two
HWDGE rings' trigger slots (the framework round-robins ~10 completion
semaphores; excess in-flight transfers serialize trigger issue) for
the warm-up x/weight stream.  A burst of zero matmuls right after the
NEFF preamble holds the PE HAM clock-gate at 8/8 until the first real
operands land, so no real matmul runs at the cold 1.2 GHz rate.
"""

import numpy as np

N = 4096
P = 128
NCORES = 8
SH = N // NCORES  # 512 output columns per core
KT = N // P  # 32 contraction tiles
MSUP = 256  # pass-1 output-row super tile (2 PSUM pairs live)
JH = KT // 2  # pass-2 j-tiles per weight half-slab

_CACHE = {}


def _build():
    import concourse.mybir as mybir
    import concourse.tile as tile
    from concourse import bacc

    f32 = mybir.dt.float32
    bf16 = mybir.dt.bfloat16
    sub = mybir.AluOpType.subtract
    add = mybir.AluOpType.add

    nc = bacc.Bacc()
    # xP[ms*128+p, k*256+c] = x[ms*256+c, k*128+p]
    xP = nc.declare_dram_parameter("xP", [N // MSUP * P, KT * MSUP], bf16,
                                   isOutput=False)
    # wrT[p, k*512+n] = W_r[c*512+n, k*128+p]   (per-core slice)
    wrT = nc.declare_dram_parameter("wrT", [P, KT * SH], bf16, isOutput=False)
    wiT = nc.declare_dram_parameter("wiT", [P, KT * SH], bf16, isOutput=False)
    # wrP[a*128+p, j*128+c] = W_r[j*128+p, a*128+c]   (tile-transposed full W)
    wrP = nc.declare_dram_parameter("wrP", [N, N], bf16, isOutput=False)
    wiP = nc.declare_dram_parameter("wiP", [N, N], bf16, isOutput=False)
    out_r = nc.declare_dram_parameter("out_r", [N, SH], f32, isOutput=True)
    out_i = nc.declare_dram_parameter("out_i", [N, SH], f32, isOutput=True)

    with tile.TileContext(nc) as tc:
        # ~6.5 us of tiny zero matmuls starting right after the NEFF
        # preamble: the PE HAM activity window stays busy until the first
        # real operands arrive (~15 us), so the real warm-up matmuls run
        # at the full 2.4 GHz clock instead of the cold 1.2 GHz rate.
        with (
            tc.tile_pool(name="warm", bufs=1) as wp,
            tc.tile_pool(name="warmp", bufs=1, space="PSUM") as wpp,
        ):
            wsb = wp.tile([P, P], bf16)
            nc.vector.memset(wsb[:], 0)
            wps = wpp.tile([P, P], f32)
            NWARM = 45
            for i in range(NWARM):
                nc.tensor.matmul(wps[:], wsb[:], wsb[:],
                                 start=i == 0, stop=i == NWARM - 1)

        # R lives in SBUF for the whole kernel: 3 x 32 KB/partition (bf16).
        # wrc0/wic0/wsc0: pass-2 a=0 first weight half-slabs (prefetched on
        # the GpSimd SWDGE queue during pass 1; wsc0 is formed on Vector).
        with tc.tile_pool(name="rres", bufs=1) as r_pool:
            rr_sb = r_pool.tile([P, KT * SH], bf16)
            ri_sb = r_pool.tile([P, KT * SH], bf16)
            rs_sb = r_pool.tile([P, KT * SH], bf16)
            wrc0 = r_pool.tile([P, JH * P], bf16)
            wic0 = r_pool.tile([P, JH * P], bf16)
            wsc0 = r_pool.tile([P, JH * P], bf16)

            # ---------- pass 1: R[:, S_c] = x @ W[S_c, :]^T ----------
            # psum[m, n] = sum_kk x[ms*256+m, k*128+kk] * W[n, k*128+kk]
            with (
                tc.tile_pool(name="wt", bufs=1) as wt_pool,
                tc.tile_pool(name="xs", bufs=3) as xs_pool,
                tc.tile_pool(name="ps1", bufs=4, space="PSUM") as ps1,
            ):
                wrT_sb = wt_pool.tile([P, KT * SH], bf16)
                wiT_sb = wt_pool.tile([P, KT * SH], bf16)
                msub = MSUP // P  # 2
                XH = KT // 2  # k-tiles per x half-tile

                def load_xh(ms, h, name):
                    xh = xs_pool.tile([P, XH * MSUP], bf16, tag="xc",
                                      name=name)
                    nc.sync.dma_start(
                        xh[:],
                        xP[ms * P : (ms + 1) * P,
                           h * XH * MSUP : (h + 1) * XH * MSUP],
                    )
                    return xh

                def wi_chunk(k0, k1):
                    nc.sync.dma_start(wiT_sb[:, k0 * SH : k1 * SH],
                                      wiT[:, k0 * SH : k1 * SH])

                # Weight preload is split across BOTH HWDGE rings so the
                # warm-up's weight demand is fed at 2-queue bandwidth: wrT on
                # the scalar ring, wiT on the sync ring interleaved with the
                # first x half-tiles (ordered so the first matmuls gate on as
                # little data as possible).
                def wr_chunk(k0, k1):
                    nc.scalar.dma_start(wrT_sb[:, k0 * SH : k1 * SH],
                                        wrT[:, k0 * SH : k1 * SH])

                # Startup schedule, need-ordered per ring.  The sync ring
                # (data from ~8.6 us) carries what the solo phase and the
                # early interleave gate on; the scalar ring (data from
                # ~11.3 us) carries the wrT bulk + mid wiT chunks; the
                # GpSimd SWDGE queue takes the latest wiT chunk.  The warm
                # matmuls consume ~240 GB/s once the fused sweep starts, so
                # neither HW ring can carry much more than ~5 MB of the
                # startup set without stalling the PE.
                x00 = load_xh(0, 0, "x00")
                wi_chunk(0, 4)
                x10 = load_xh(1, 0, "x10")
                wi_chunk(8, 16)
                x01 = load_xh(0, 1, "x01")
                wr_chunk(0, 4)
                wr_chunk(4, 8)
                nc.scalar.dma_start(wiT_sb[:, 4 * SH : 8 * SH],
                                    wiT[:, 4 * SH : 8 * SH])
                wr_chunk(8, 16)
                wr_chunk(16, 24)
                nc.scalar.dma_start(wiT_sb[:, 16 * SH : 24 * SH],
                                    wiT[:, 16 * SH : 24 * SH])
                wr_chunk(24, 32)
                nc.scalar.dma_start(wiT_sb[:, 24 * SH : 32 * SH],
                                    wiT[:, 24 * SH : 32 * SH])
                # prefetch pass-2 a=0 first half-slabs; queued at the TAIL
                # of the scalar HW ring so they land after the pass-1
                # weights and never contend with the startup stream (any
                # earlier queue — including the GpSimd SWDGE one — starts
                # transferring immediately and steals HBM bandwidth from
                # the chunks the first matmuls gate on).  wsc0 = wrc0 +
                # wic0 on Vector replaces the third prefetch stream.
                nc.scalar.dma_start(wrc0[:], wrP[0:P, 0 : JH * P])
                nc.scalar.dma_start(wic0[:], wiP[0:P, 0 : JH * P])
                nc.vector.tensor_tensor(wsc0[:], wrc0[:], wic0[:], add)

                def alloc_acc():
                    acc_r = [
                        ps1.tile([P, SH], f32, tag="ps_r", name=f"accr{_s}")
                        for _s in range(msub)
                    ]
                    acc_i = [
                        ps1.tile([P, SH], f32, tag="ps_i", name=f"acci{_s}")
                        for _s in range(msub)
                    ]
                    return acc_r, acc_i

                def k_step(xh, acc_r, acc_i, k):
                    kk = k % XH
                    first, last = k == 0, k == KT - 1
                    for s in range(msub):
                        lhs = xh[:, kk * MSUP + s * P : kk * MSUP + (s + 1) * P]
                        nc.tensor.matmul(
                            acc_r[s][:], lhs, wrT_sb[:, k * SH : (k + 1) * SH],
                            start=first, stop=last,
                        )
                        nc.tensor.matmul(
                            acc_i[s][:], lhs, wiT_sb[:, k * SH : (k + 1) * SH],
                            start=first, stop=last,
                        )

                def evac(ms, acc_r, acc_i):
                    for s in range(msub):
                        mt = ms * msub + s
                        sl = slice(mt * SH, (mt + 1) * SH)
                        nc.scalar.copy(rr_sb[:, sl], acc_r[s][:])
                        nc.vector.tensor_copy(ri_sb[:, sl], acc_i[s][:])
                        nc.gpsimd.tensor_add(
                            rs_sb[:, sl], rr_sb[:, sl], ri_sb[:, sl]
                        )

                # --- fused warm-up: ms=0,1 share one k-interleaved sweep
                # (8 PSUM banks), halving the weight-DMA demand per unit of
                # compute while the PE ramps and the 8 MB of weights stream
                # in.  ms0 runs k=0..3 solo first so ms1's x half-tile has
                # time to arrive behind it on the sync ring.
                acc0 = alloc_acc()
                acc1 = alloc_acc()
                for k in range(0, 4):
                    k_step(x00, *acc0, k)
                for k in range(0, XH - 4):
                    k_step(x10, *acc1, k)
                    k_step(x00, *acc0, k + 4)
                x11 = load_xh(1, 1, "x11")
                for k in range(XH - 4, XH):
                    k_step(x10, *acc1, k)
                for k in range(XH, KT):
                    k_step(x01, *acc0, k)
                for k in range(XH, KT):
                    k_step(x11, *acc1, k)
                evac(0, *acc0)
                evac(1, *acc1)

                for ms in range(2, N // MSUP):  # 14 regular iterations
                    xh0 = load_xh(ms, 0, "xh0")
                    xh1 = load_xh(ms, 1, "xh1")
                    if ms < N // MSUP - 1:
                        acc_r, acc_i = alloc_acc()
                        for k in range(KT):
                            k_step(xh0 if k < XH else xh1, acc_r, acc_i, k)
                        evac(ms, acc_r, acc_i)
                    else:
                        # final iteration runs as two 1-subtile generations
                        # (2 PSUM banks each, same matmul columns) so six
                        # banks are already free when pass 2's t-groups
                        # start — removes the pass-transition PSUM handoff
                        for half in range(msub):
                            ar = ps1.tile([P, SH], f32, tag="ps_r",
                                          name=f"accrh{half}")
                            ai = ps1.tile([P, SH], f32, tag="ps_i",
                                          name=f"accih{half}")
                            for k in range(KT):
                                xh = xh0 if k < XH else xh1
                                kk = k % XH
                                lhs = xh[:, kk * MSUP + half * P
                                         : kk * MSUP + (half + 1) * P]
                                nc.tensor.matmul(
                                    ar[:], lhs,
                                    wrT_sb[:, k * SH : (k + 1) * SH],
                                    start=k == 0, stop=k == KT - 1,
                                )
                                nc.tensor.matmul(
                                    ai[:], lhs,
                                    wiT_sb[:, k * SH : (k + 1) * SH],
                                    start=k == 0, stop=k == KT - 1,
                                )
                            mt = ms * msub + half
                            sl = slice(mt * SH, (mt + 1) * SH)
                            nc.scalar.copy(rr_sb[:, sl], ar[:])
                            nc.vector.tensor_copy(ri_sb[:, sl], ai[:])
                            nc.gpsimd.tensor_add(
                                rs_sb[:, sl], rr_sb[:, sl], ri_sb[:, sl]
                            )

            # ---------- pass 2: C[:, S_c] = W^T @ R (Karatsuba) ----------
            # t1[a,b] = sum_j wr[j,a] rr[j,b]; t2: wi,ri; t3: ws,rs
            # weight chunks stream as half-slabs (j 0..15 / 16..31);
            # ws = wr + wi is formed on the Vector engine as slabs land,
            # halving each HWDGE ring's pass-2 load (1 MB per a-tile).
            with (
                tc.tile_pool(name="ws", bufs=3) as ws_pool,
                tc.tile_pool(name="ev2", bufs=3) as ev2_pool,
                tc.tile_pool(name="ps2", bufs=2, space="PSUM") as ps2,
            ):
                for a in range(N // P):  # 32
                    asl = slice(a * P, (a + 1) * P)
                    halves = []
                    for hj in range(2):
                        if a == 0 and hj == 0:
                            halves.append((wrc0, wic0, wsc0))
                            continue
                        csl = slice(hj * JH * P, (hj + 1) * JH * P)
                        wrc = ws_pool.tile([P, JH * P], bf16, tag="wrc",
                                           name=f"wrc{hj}")
                        wic = ws_pool.tile([P, JH * P], bf16, tag="wic",
                                           name=f"wic{hj}")
                        wsc = ws_pool.tile([P, JH * P], bf16, tag="wsc",
                                           name=f"wsc{hj}")
                        nc.sync.dma_start(wrc[:], wrP[asl, csl])
                        nc.scalar.dma_start(wic[:], wiP[asl, csl])
                        nc.vector.tensor_tensor(wsc[:], wrc[:], wic[:], add)
                        halves.append((wrc, wic, wsc))
                    # The last a-tile is split into two 256-wide column
                    # groups so the first group's combines + stores overlap
                    # the second group's matmuls — the full-width tail chain
                    # (combine ~2.4 us + store drain) otherwise sits fully
                    # after the final matmul.
                    last_a = a == N // P - 1
                    for cg, cw in ([(0, SH)] if not last_a
                                   else [(0, SH // 2), (1, SH // 2)]):
                        t1 = ps2.tile([P, cw], f32, tag="t1")
                        t2 = ps2.tile([P, cw], f32, tag="t2")
                        t3 = ps2.tile([P, cw], f32, tag="t3")
                        for j in range(KT):
                            first, last = j == 0, j == KT - 1
                            hj, jj = divmod(j, JH)
                            wrc, wic, wsc = halves[hj]
                            wsl = slice(jj * P, (jj + 1) * P)
                            r0 = j * SH + cg * cw
                            rsl = slice(r0, r0 + cw)
                            nc.tensor.matmul(
                                t1[:], wrc[:, wsl], rr_sb[:, rsl],
                                start=first, stop=last,
                            )
                            nc.tensor.matmul(
                                t2[:], wic[:, wsl], ri_sb[:, rsl],
                                start=first, stop=last,
                            )
                            nc.tensor.matmul(
                                t3[:], wsc[:, wsl], rs_sb[:, rsl],
                                start=first, stop=last,
                            )
                        s1 = ev2_pool.tile([P, cw], f32, tag="s1")
                        cr = ev2_pool.tile([P, cw], f32, tag="cr")
                        ci = ev2_pool.tile([P, cw], f32, tag="ci")
                        nc.scalar.copy(s1[:], t1[:])
                        nc.vector.tensor_tensor(cr[:], s1[:], t2[:], sub)
                        nc.vector.tensor_tensor(ci[:], t3[:], s1[:], sub)
                        nc.vector.tensor_tensor(ci[:], ci[:], t2[:], sub)
                        # outputs go on the Pool/SWDGE path: their triggers
                        # wait on cr/ci, and on the strict-FIFO ACT ring that
                        # wait would block the next wic prefetch trigger.
                        # The final group uses the (now idle) sync HWDGE ring
                        # instead — SWDGE descriptor generation (~7 us) would
                        # otherwise sit on the critical path at the tail.
                        oeng = nc.sync if (last_a and cg == 1) else nc.gpsimd
                        osl = slice(cg * cw, cg * cw + cw)
                        oeng.dma_start(out_r[asl, osl], cr[:])
                        oeng.dma_start(out_i[asl, osl], ci[:])

    nc.finalize()
    return nc


def _get_nc():
    if "nc" not in _CACHE:
        _CACHE["nc"] = _build()
    return _CACHE["nc"]


def _prep_inputs(x, W_r, W_i):
    from ml_dtypes import bfloat16

    x = np.asarray(x, dtype=np.float32)
    Wr = np.asarray(W_r, dtype=np.float32)
    Wi = np.asarray(W_i, dtype=np.float32)

    # xP[ms*128+p, k*256+c] = x[ms*256+c, k*128+p]
    xP = np.ascontiguousarray(
        x.reshape(N // MSUP, MSUP, KT, P).transpose(0, 3, 2, 1)
        .reshape(N // MSUP * P, KT * MSUP)
    ).astype(bfloat16)

    # wP[a*128+p, j*128+c] = W[j*128+p, a*128+c]
    def p2(W):
        return np.ascontiguousarray(
            W.reshape(KT, P, KT, P).transpose(2, 1, 0, 3).reshape(N, N)
        ).astype(bfloat16)

    # wT_c[p, k*512+n] = W[c*512+n, k*128+p]
    def p1(W, c):
        blk = W[c * SH : (c + 1) * SH, :].T  # [4096 (k), 512 (n)]
        return np.ascontiguousarray(
            blk.reshape(KT, P, SH).transpose(1, 0, 2).reshape(P, KT * SH)
        ).astype(bfloat16)

    wrP, wiP = p2(Wr), p2(Wi)
    in_maps = []
    for c in range(NCORES):
        in_maps.append(
            {
                "xP": xP,
                "wrT": p1(Wr, c),
                "wiT": p1(Wi, c),
                "wrP": wrP,
                "wiP": wiP,
            }
        )
    return in_maps


def kernel(x, W_r, W_i, **run_kwargs):
    from concourse.bass_utils import run_bass_kernel_spmd

    nc = _get_nc()
    in_maps = _prep_inputs(x, W_r, W_i)
    out = run_bass_kernel_spmd(nc, in_maps, list(range(NCORES)), **run_kwargs)
    res = out.results

    full = np.empty((N, 2 * N), dtype=np.float32)
    for c in range(NCORES):
        full[:, c * SH : (c + 1) * SH] = res[c]["out_r"]
        full[:, N + c * SH : N + (c + 1) * SH] = res[c]["out_i"]
    if run_kwargs:
        _CACHE["last_result"] = out
    return full

